# revision 16
# baseline (speedup 1.0000x reference)
"""Trainium2 Bass kernel for nn_DissipativeRINN.

Math per time step t (per sample):
    w = fixed_point(w -> tanh(Cv x + Dvy y + Dvw w))   [NITER iters from w=0]
    u = Cu x + Duw w + Duy y
    x <- x + DT*(A x + Bw w + By y)
Output: concat([u_seq, broadcast(log_stds)], axis=-1)

Device strategy (pure data parallel, 8 cores x 512 batch):
  - Everything stored TRANSPOSED on chip: xyT [32,cols] (x rows 0:16, y rows
    16:32), wT [128,cols]; obs pre-transposed per core on host to [T,16,512].
  - The 512-sample batch is split into NS=2 independent 256-col streams,
    interleaved at the iteration level: stream B's matmuls execute on PE
    while stream A's tanh runs on ScalarE (and vice versa), so the serial
    matmul->tanh->matmul dependency chain of the fixed-point loop never
    stalls PE. 256 cols keeps f32r matmuls at the 1 cycle/col fast path.
  - Per fixed-point iteration per stream:
        psum  = [Cv|Dvy].T.T @ xyT   (K=32 matmul, start)
        psum += Dvw @ wT             (K=128 matmul, stop)
        wT    = tanh(psum)           (ScalarE, PSUM->SBUF)
  - Step end, one accumulation per stream computes both x_next and u:
        psum_xu = Gxy.T @ xyT + Gw.T @ wT   rows 0:16 = x_next^T, 32:40 = u^T
    with DT and the Euler identity folded into Gxy/Gw on host; a DVE copy
    moves x into the next step's xy tile, u DMAs out via a staging tile.
    The other stream's fixed-point matmuls overlap these copies/DMAs.
  - Precision: ALL matmuls in float32r (1 cycle/col on PE vs 4 for fp32;
    ~1.5e-4 rel noise on HW). NITER=3 truncates the reference's 30
    fixed-point iterations; contraction ~0.23/iter puts truncation at
    ~4.5e-3 u-rel. u ships in bf16 (+~2e-3 max elementwise rounding).
    Total rel err ~2.0e-3 full-output, 10x under the 2e-2 gate.
  - xy ping-pong is a ring of 3 so the y-prefetch DMA isn't serialized
    behind the previous step's matmul reads.
  - Device time ~260 us/rollout (in-NEFF repetition-scaling measurement):
    PE-bound at ~2 us/step with the tanh chain fully hidden by the stream
    interleave. The sustained per-exec wall time through the axon PJRT
    pipeline adds a serial ~0.88 ms/exec backend stage (infra-fixed: a
    zero-work NEFF measures the same; independent of instruction count,
    donation, output dtype, device count, and client threading), so the
    test.py-printed number (~0.96 ms) sits ~9% above the floor of this
    environment with most of the device time overlapped.
"""

import numpy as np

import concourse.bass as bass
import concourse.bacc as bacc
import concourse.mybir as mybir
import concourse.tile as tile
from concourse.bass_utils import run_bass_kernel_spmd

# Problem constants (hardcoded per harness contract)
BATCH, T, S, N, IN, OUT = 4096, 128, 16, 128, 16, 8
NCORES = 8
BC = BATCH // NCORES          # 512 samples per core
NITER = 3                     # fixed-point iterations: truncation vs the
                              # reference's 30 iters ~4.5e-3 u-rel / ~1.9e-3
                              # full-output rel (contraction ~0.23/iter), 10x
                              # under the 2e-2 gate
NS = 2                        # interleaved batch streams per core
DT = np.float32(0.01)
XU_M = 40                     # xu psum rows: 0:16 x_next, 32:40 u (32-aligned)
UOFF = 32

# const blob column layout: [128, CBLOB]
C_DVWT = 0                    # [128, 0:128]   Dvw^T
C_GW = 128                    # [128, 128:168] Gw
C_CVDVY = 168                 # [32, 168:296]  [Cv|Dvy] stacked (rows 0:32)
C_GXY = 296                   # [32, 296:336]  Gxy       (rows 0:32)
C_XY0 = 336                   # [32, 336:848]  initial [x0;y0]^T
CBLOB = C_XY0 + BC

F32 = mybir.dt.float32
F32R = mybir.dt.float32r
BF16 = mybir.dt.bfloat16
TANH = mybir.ActivationFunctionType.Tanh
IDENT = mybir.ActivationFunctionType.Identity


def build(nsteps=T, niter=NITER, bc=BC, ns=NS, reps=1, u16=True):
    """Build the per-core Bass program. Same program runs on all 8 cores."""
    hb = bc // ns
    udt = BF16 if u16 else F32
    nc = bacc.Bacc("TRN2", target_bir_lowering=False, debug=False)

    obs_d = nc.dram_tensor("obs_t", [nsteps, IN, bc], F32R, kind="ExternalInput")
    blob_d = nc.dram_tensor("blob", [N, C_XY0 + bc], F32R, kind="ExternalInput")
    u_d = nc.dram_tensor("u_t", [nsteps, OUT, bc], udt, kind="ExternalOutput")

    with tile.TileContext(nc) as tc:
        with (
            tc.tile_pool(name="const", bufs=1) as constp,
            tc.tile_pool(name="state", bufs=1) as statep,
            tc.tile_pool(name="wps", bufs=4, space=bass.MemorySpace.PSUM) as wps,
            tc.tile_pool(name="xups", bufs=2, space=bass.MemorySpace.PSUM) as xups,
            tc.tile_pool(name="ustg", bufs=4) as ustgp,
        ):
            blob = constp.tile([N, C_XY0 + bc], F32R, tag="blob")
            nc.sync.dma_start(blob[:], blob_d[:])
            dvwT = blob[:, C_DVWT:C_DVWT + N]
            gw = blob[:, C_GW:C_GW + XU_M]
            cvdvy = blob[0:S + IN, C_CVDVY:C_CVDVY + N]
            gxy = blob[0:S + IN, C_GXY:C_GXY + XU_M]

            # Per-stream persistent state: w iterate + ring-of-3 xy tiles
            # (ring so the y-prefetch DMA isn't serialized behind the
            # previous step's matmul reads). xy rows 0:16 = x^T, 16:32 = y^T.
            NBUF = 3
            ws, xyss = [], []
            for s in range(ns):
                ws.append(statep.tile([N, hb], F32R, tag=f"w{s}",
                                      name=f"w{s}"))
                xyss.append([
                    statep.tile([S + IN, hb], F32R, tag=f"xy{s}_{i}",
                                name=f"xy{s}_{i}")
                    for i in range(NBUF)
                ])

            def scols(s):
                return slice(s * hb, (s + 1) * hb)

            for rt in range(reps * nsteps):
                # reps>1 re-runs the whole rollout in-NEFF (timing probe:
                # device time scales by reps, RPC overhead does not)
                t = rt % nsteps
                curs = [blob[0:S + IN, C_XY0 + s * hb:C_XY0 + (s + 1) * hb]
                        if t == 0 else xyss[s][t % NBUF] for s in range(ns)]
                nxts = [xyss[s][(t + 1) % NBUF] for s in range(ns)]
                if t + 1 < nsteps:
                    for s in range(ns):
                        nc.sync.dma_start(nxts[s][S:S + IN, :],
                                          obs_d[t + 1][:, scols(s)])

                for k in range(niter):
                    for s in range(ns):
                        ps = wps.tile([N, hb], F32, tag="ps")
                        if k == 0:
                            # w starts at 0: first iterate is tanh(b)
                            nc.tensor.matmul(ps[:], cvdvy, curs[s][:],
                                             start=True, stop=True)
                        else:
                            nc.tensor.matmul(ps[:], cvdvy, curs[s][:],
                                             start=True, stop=False)
                            nc.tensor.matmul(ps[:], dvwT, ws[s][:],
                                             start=False, stop=True)
                        nc.scalar.activation(ws[s][:], ps[:], TANH)

                # x_next and u in one accumulation per stream:
                # rows 0:16 x_next, 32:40 u. (Issuing the gxy pass early,
                # inside the last iteration, measured ~2% WORSE — an open
                # psum accumulation across intervening matmuls serializes.)
                for s in range(ns):
                    pxu = xups.tile([XU_M, hb], F32, tag="pxu")
                    nc.tensor.matmul(pxu[:], gxy, curs[s][:],
                                     start=True, stop=False)
                    nc.tensor.matmul(pxu[:], gw, ws[s][:],
                                     start=False, stop=True)
                    if t + 1 < nsteps:
                        # x-copy on ScalarE (idle once its tanh work is done)
                        # so the xu->copy->next-b boundary chain doesn't wait
                        # behind DVE, which handles the u staging in parallel.
                        # Aligned rows; never touches the prefetched y rows.
                        nc.scalar.activation(nxts[s][0:S, :], pxu[0:S, :],
                                             IDENT)
                    ustg = ustgp.tile([OUT, hb], udt, tag="ustg")
                    nc.vector.tensor_copy(ustg[:], pxu[UOFF:UOFF + OUT, :])
                    nc.sync.dma_start(u_d[t][:, scols(s)], ustg[:])

    nc.compile()
    return nc


def prep_inputs(obs, state0, A, Bw, By, Cv, Dvw, Dvy, Cu, Duw, Duy, nsteps=T, bc=BC):
    """Host-side shard + transpose. Returns in_maps for run_bass_kernel_spmd."""
    obs = np.ascontiguousarray(obs, dtype=np.float32)
    state0 = np.ascontiguousarray(state0, dtype=np.float32)

    A2 = (np.eye(S, dtype=np.float32) + DT * A).astype(np.float32)
    blob = np.zeros((N, C_XY0 + bc), dtype=np.float32)
    blob[:, C_DVWT:C_DVWT + N] = Dvw.T
    blob[:, C_GW:C_GW + S] = (DT * Bw).T
    blob[:, C_GW + UOFF:C_GW + UOFF + OUT] = Duw.T
    blob[0:S, C_CVDVY:C_CVDVY + N] = Cv.T
    blob[S:S + IN, C_CVDVY:C_CVDVY + N] = Dvy.T
    blob[0:S, C_GXY:C_GXY + S] = A2.T
    blob[S:S + IN, C_GXY:C_GXY + S] = (DT * By).T
    blob[0:S, C_GXY + UOFF:C_GXY + UOFF + OUT] = Cu.T
    blob[S:S + IN, C_GXY + UOFF:C_GXY + UOFF + OUT] = Duy.T

    ncores = obs.shape[0] // bc
    in_maps = []
    for c in range(ncores):
        rows = slice(c * bc, (c + 1) * bc)
        obs_t = np.ascontiguousarray(
            obs[rows, :nsteps, :].transpose(1, 2, 0))                 # [T,16,bc]
        cblob = blob.copy()
        cblob[0:S, C_XY0:C_XY0 + bc] = state0[rows].T
        cblob[S:S + IN, C_XY0:C_XY0 + bc] = obs_t[0]
        in_maps.append({"obs_t": obs_t, "blob": cblob})
    return in_maps


_CACHE = {}


def run(inputs, nsteps=T, niter=NITER, ns=NS, u16=True, trace=False,
        trace_kwargs=None):
    """Shard inputs, run on 8 cores, return (full_output, BassKernelResults)."""
    key = (nsteps, niter, ns, u16)
    if key not in _CACHE:
        _CACHE[key] = build(nsteps=nsteps, niter=niter, ns=ns, u16=u16)
    nc = _CACHE[key]

    inputs = {k: np.asarray(v, dtype=np.float32) for k, v in inputs.items()}
    in_maps = prep_inputs(
        inputs["obs"], inputs["state0"], inputs["A"], inputs["Bw"], inputs["By"],
        inputs["Cv"], inputs["Dvw"], inputs["Dvy"], inputs["Cu"], inputs["Duw"],
        inputs["Duy"], nsteps=nsteps,
    )
    res = run_bass_kernel_spmd(
        nc, in_maps, core_ids=list(range(NCORES)), trace=trace,
        **(trace_kwargs or {}),
    )

    log_stds = np.asarray(inputs["log_stds"], dtype=np.float32)
    out = np.empty((BATCH, nsteps, 2 * OUT), dtype=np.float32)
    for c in range(NCORES):
        u_t = res.results[c]["u_t"]                       # [nsteps, OUT, bc]
        out[c * BC:(c + 1) * BC, :, :OUT] = (
            u_t.astype(np.float32).transpose(2, 0, 1))
    out[:, :, OUT:] = log_stds                            # broadcast exact values
    return out, res


def kernel(**inputs) -> np.ndarray:
    out, _ = run(inputs)
    return out
